# revision 4
# baseline (speedup 1.0000x reference)
"""Trainium2 Bass kernel for a binary-conv ResNet BasicBlock (training-mode BN).

Reference computation (per nn_BasicBlock_52158082843180):
    out = sign( BN2( conv3x3(sign(BN1(conv3x3(x, sign(w1)))), sign(w2)) ) + x )
with training-mode BatchNorm (batch stats over (N,H,W), biased var, eps=1e-5).

Strategy (8 NeuronCores, data-parallel over batch N=32 -> 4 images/core):
  * conv3x3 as 9 shift-matmuls on TensorE in a 58x58 zero-padded layout.
  * conv1 inputs split on the host into fp16 hi + fp16 lo (exact to ~2^-24);
    conv2 uses e4m3 DoubleRow matmuls (exact +-1 arithmetic).
  * sync-BN via AllReduce of per-core channel sums. When beta1 == 0 and
    gamma1 > 0 (the shipped inputs), sign(BN1(c)) == sign(c - mean), so BN1
    needs NO variance: conv1 skips the whole sum-of-squares pass and the BN1
    AllReduce carries sums only.
  * conv2's sum-of-squares runs on DVE, keeping
    ScalarE free for PSUM evacuation so the PE isn't gated on psum recycling.
  * binary activations live in per-image tiles so conv2's first matmuls start
    as soon as image 0's sign pass lands (not after all four).
  * the final residual+sign tail is 2 passes (DVE fused scale+add, ACT Sign
    with bias) and writes fp8 (+-1 exact); the residual streams in as fp16.
    Host upcasts the output to f32.

kernel(**inputs) takes the full unsharded inputs and returns the full output.
"""

import os
import sys

for _p in ("/root/.axon_site/_ro/trn_rl_repo", "/opt/trn_rl_repo"):
    if os.path.isdir(_p) and _p not in sys.path:
        sys.path.append(_p)

import numpy as np
from contextlib import ExitStack

import concourse.bass as bass
import concourse.bacc as bacc
import concourse.tile as tile
from concourse import mybir, bass_utils

# ---------------------------------------------------------------- constants
N_CORES = 8
B, C, H, W = 32, 256, 56, 56
BSH = B // N_CORES            # images per core
HP, WP = H + 2, W + 2         # padded spatial
FLAT = HP * WP                # 3364 padded pixels per image
NCH = C // 128                # channel chunks of 128 (=2)
NTAP = 3                      # 3x3 kernel
NQ = 4                        # quarters per image (14 output rows each)
RQ = H // NQ                  # output rows per quarter (14)
NCK = 2                       # psum chunks per quarter
RCK = RQ // NCK               # output rows per psum chunk (7)
CKW = RCK * WP                # psum chunk width incl. junk cols (406)
VCK = RCK * W                 # valid elements per chunk (392)
QROWS = RQ + 2                # padded input rows needed per quarter (16)
EPS = 1e-5

F32 = mybir.dt.float32
FP16 = mybir.dt.float16
BF16 = mybir.dt.bfloat16
BA_DT = mybir.dt.float8e4    # binary activation storage (+-1 exact)
HCK = CKW // 2               # DoubleRow half-chunk output width (203)

def _np_dt(dt):
    return np.dtype(mybir.dt.np(dt))


# ---------------------------------------------------------------- program
def build_nc(n_cores=N_CORES, fast_bn1=True, loops=1):
    """loops>1 replicates the whole computation serially inside one NEFF
    (each iteration's first reads depend on the previous iteration's last
    output tile) -- used only for timing: per-exec time =
    (T_loops - T_1) / (loops - 1), cancelling dispatch+I/O overhead.
"""
    nc = bacc.Bacc(
        "TRN2",
        target_bir_lowering=False,
        debug=False,
        enable_asserts=False,
        num_devices=n_cores,
    )
    # per-core DRAM I/O
    xh = nc.dram_tensor("x_hi", [BSH, NCH, 128, FLAT], FP16, kind="ExternalInput").ap()
    xl = nc.dram_tensor("x_lo", [BSH, NCH, 128, FLAT], FP16, kind="ExternalInput").ap()
    xr = nc.dram_tensor("x_res", [BSH, NCH, 128, H * W], F32, kind="ExternalInput").ap()
    w1 = nc.dram_tensor("w1t", [NCH, 128, 9, C], FP16, kind="ExternalInput").ap()
    w2 = nc.dram_tensor("w2t", [128, NCH, 9, C], BA_DT, kind="ExternalInput").ap()
    gb = nc.dram_tensor("gb", [128, 4, NCH], F32, kind="ExternalInput").ap()
    out = nc.dram_tensor("out", [BSH, NCH, 128, H * W], BA_DT, kind="ExternalOutput").ap()
    xsp = [xh, xl]

    with tile.TileContext(nc) as tc, ExitStack() as ctx:
        wpool = ctx.enter_context(tc.tile_pool(name="weights", bufs=1))
        big = ctx.enter_context(tc.tile_pool(name="big", bufs=1))
        xqp = ctx.enter_context(tc.tile_pool(name="xq", bufs=1))
        psum = ctx.enter_context(tc.tile_pool(name="psum", bufs=8, space="PSUM"))
        stp = ctx.enter_context(tc.tile_pool(name="stats", bufs=1))
        scrp = ctx.enter_context(tc.tile_pool(name="scr", bufs=2))
        smp = ctx.enter_context(tc.tile_pool(name="small", bufs=1))
        finp = ctx.enter_context(tc.tile_pool(name="fin", bufs=8))
        fin8p = ctx.enter_context(tc.tile_pool(name="fin8", bufs=4))
        finbp = ctx.enter_context(tc.tile_pool(name="finb", bufs=4))
        dram = ctx.enter_context(tc.tile_pool(name="dram", bufs=1, space="DRAM"))

        # ---- persistent tiles
        w1_sb = [wpool.tile([128, 9, C], FP16, tag=f"w1_{c}", name=f"w1_{c}") for c in range(NCH)]
        for c in range(NCH):
            nc.scalar.dma_start(out=w1_sb[c][:], in_=w1[c])
        w2_sb = wpool.tile([128, NCH, 9, C], BA_DT, tag="w2", name="w2")

        # out_sb holds conv1 output (valid pixels only, f32), later reused
        # in-place for conv2 output.
        out_sb = [big.tile([128, BSH, H * W], F32, tag=f"out_{c}", name=f"out_{c}") for c in range(NCH)]
        # binary activations, one padded-layout tile per image so conv2 can
        # start on image 0 while later images' sign passes still run; +1
        # guard element at each end of each cin-chunk plane.
        ba_im = [big.tile([128, NCH, FLAT + 2], BA_DT, tag=f"ba{i}", name=f"ba{i}")
                 for i in range(BSH)]
        for t in ba_im:
            nc.gpsimd.memset(t[:], 0.0)

        # x-quarter staging: 2 manually-rotated buffer sets; only the 2 guard
        # elements (read into junk output columns) need a one-time zero.
        QW = QROWS * WP + 2
        xq_bufs = [[[xqp.tile([128, QW], FP16,
                              tag=f"xq_{b}_{s}_{cic}", name=f"xq_{b}_{s}_{cic}")
                     for cic in range(NCH)] for s in range(2)]
                   for b in range(2)]
        for bset in xq_bufs:
            for row in bset:
                for t in row:
                    nc.vector.memset(t[:, 0:1], 0.0)
                    nc.vector.memset(t[:, QW - 1:QW], 0.0)
        gb_sb = smp.tile([128, 4, NCH], F32, tag="gb", name="gb")
        nc.scalar.dma_start(out=gb_sb[:], in_=gb)
        eps_sb = smp.tile([128, 1], F32, tag="eps", name="eps")
        nc.vector.memset(eps_sb[:], EPS)
        # warm the ACT Sqrt table so the post-AllReduce BN-scale chain does
        # not stall on a lazy ACT_TABLE_LOAD at the first Sqrt use
        warm = smp.tile([128, 1], F32, tag="warm", name="warm")
        nc.scalar.activation(out=warm[:], in_=eps_sb[:],
                             func=mybir.ActivationFunctionType.Sqrt,
                             bias=eps_sb[:], scale=1.0)

        def conv_pass(conv_idx, cocs, need_sq):
            """Emit one conv's matmuls + psum->sbuf copies + per-chunk
            sum (and optionally sumsq) stats for the given output-channel
            chunks. conv1 reads streamed x quarters; conv2 reads ba_im."""
            is1 = conv_idx == 1
            nchunk = BSH * NQ * NCK
            sums = {c: stp.tile([128, nchunk], F32, tag=f"sum_{c}",
                                name=f"sum{conv_idx}_{c}") for c in cocs}
            sqs = {c: stp.tile([128, nchunk], F32, tag=f"sq_{c}",
                               name=f"sq{conv_idx}_{c}") for c in cocs} if need_sq else None
            for img in range(BSH):
                for q in range(NQ):
                    if is1:
                        # stream the 16 padded input rows of this quarter
                        xq = xq_bufs[(img * NQ + q) % 2]
                        for s in range(2):
                            for cic in range(NCH):
                                nc.sync.dma_start(
                                    out=xq[s][cic][:, 1:1 + QROWS * WP],
                                    in_=xsp[s][img, cic, :,
                                               q * RQ * WP: q * RQ * WP + QROWS * WP],
                                )
                    for coc in cocs:
                        pt = [psum.tile([128, CKW], F32, tag="psum", name="pt") for _ in range(NCK)]
                        cosl = slice(coc * 128, (coc + 1) * 128)
                        started = [False] * NCK
                        if is1:
                            # hi/lo fp16 passes: full-chunk matmuls per cin chunk
                            for ky in range(NTAP):
                                for kx in range(NTAP):
                                    tap = ky * NTAP + kx
                                    for cic in range(NCH):
                                        lhsT = w1_sb[cic][:, tap, cosl]
                                        for s in range(2):
                                            for ck in range(NCK):
                                                # xq guard(+1) and tap col(-1) cancel
                                                off = (7 * ck + ky) * WP + kx
                                                last = (tap == 8
                                                        and cic == NCH - 1 and s == 1)
                                                nc.tensor.matmul(
                                                    pt[ck][:], lhsT,
                                                    xq[s][cic][:, off: off + CKW],
                                                    start=not started[ck], stop=last)
                                                started[ck] = True
                        else:
                            # conv2: e4m3 DoubleRow, both cin chunks per MM.
                            # One full-width matmul per (tap, ck): FD=406
                            # keeps DoubleRow above its FD>=256 sweet spot.
                            base = 1 + q * RQ * WP
                            for ky in range(NTAP):
                                for kx in range(NTAP):
                                    tap = ky * NTAP + kx
                                    lhsT = w2_sb[:, :, tap, cosl]
                                    for ck in range(NCK):
                                        off = base + (7 * ck + ky) * WP + kx - 1
                                        nc.tensor.matmul(
                                            pt[ck][:],
                                            lhsT,
                                            ba_im[img][:, :, off: off + CKW],
                                            perf_mode=mybir.MatmulPerfMode.DoubleRow,
                                            start=(tap == 0),
                                            stop=(tap == 8))
                        # evacuate psum (valid cols only); ScalarE copy also
                        # emits the chunk row-sum; DVE computes sum of squares
                        # in one fused pass when the variance is needed
                        for ck in range(NCK):
                            ci = q * NCK + ck
                            sidx = (img * NQ + q) * NCK + ck
                            dst = out_sb[coc][:, img, ci * VCK:(ci + 1) * VCK]
                            dst3 = dst.rearrange("p (r w) -> p r w", w=W)
                            src3 = pt[ck][:].rearrange("p (r w) -> p r w", w=WP)[:, :, 1:1 + W]
                            nc.scalar.activation(
                                out=dst3, in_=src3,
                                func=mybir.ActivationFunctionType.Copy,
                                accum_out=sums[coc][:, sidx:sidx + 1])
                            if need_sq:
                                scr = scrp.tile([128, VCK], F32, tag="scr", name="scr")
                                nc.vector.tensor_mul(scr[:], dst, dst)
                                nc.vector.reduce_sum(sqs[coc][:, sidx:sidx + 1],
                                                     scr[:],
                                                     axis=mybir.AxisListType.X)
            return sums, sqs

        def sync_bn(stats, tag, bn_idx, cocs):
            """AllReduce stats over `cocs`. Full mode: returns per-channel
            (s_t, t_t) scale/bias. Mean-only mode (BN1 fast path): returns
            (None, t_t) with t_t = -global_mean."""
            sums, sqs = stats
            mean_only = sqs is None
            nco = len(cocs)
            width = 1 if mean_only else 2
            pay = smp.tile([128, nco, width], F32, tag=f"pay{tag}", name=f"pay{tag}")
            for i, coc in enumerate(cocs):
                nc.vector.reduce_sum(pay[:, i, 0:1], sums[coc][:],
                                     axis=mybir.AxisListType.X)
                if not mean_only:
                    nc.vector.reduce_sum(pay[:, i, 1:2], sqs[coc][:],
                                         axis=mybir.AxisListType.X)
            cin = dram.tile([128, nco * width], F32, tag=f"cin{tag}", name=f"cin{tag}")
            cout_ = dram.tile([128, nco * width], F32, tag=f"cout{tag}",
                              addr_space="Shared" if n_cores % 2 == 0 else "Local",
                              name=f"ccout{tag}")
            # stats payloads go out on the ACT hwdge queue so they never sit
            # behind bulk image DMAs on the SP queue
            nc.scalar.dma_start(out=cin[:], in_=pay[:].rearrange("p a b -> p (a b)"))
            nc.gpsimd.collective_compute(
                "AllReduce", mybir.AluOpType.add,
                replica_groups=[list(range(n_cores))],
                ins=[cin.opt()], outs=[cout_.opt()],
            )
            ars = smp.tile([128, nco, width], F32, tag=f"ars{tag}", name=f"ars{tag}")
            nc.scalar.dma_start(out=ars[:].rearrange("p a b -> p (a b)"), in_=cout_[:])
            inv = 1.0 / (BSH * n_cores * H * W)
            t_t = smp.tile([128, nco], F32, tag=f"t{tag}", name=f"t{tag}")
            if mean_only:
                # t = -mean; ba = sign(conv - mean)
                nc.vector.tensor_scalar_mul(t_t[:], ars[:, :, 0], -inv)
                return None, t_t
            gm = smp.tile([128, nco], F32, tag=f"gm{tag}", name=f"gm{tag}")
            gv = smp.tile([128, nco], F32, tag=f"gv{tag}", name=f"gv{tag}")
            s_t = smp.tile([128, nco], F32, tag=f"s{tag}", name=f"s{tag}")
            nc.vector.tensor_scalar_mul(gm[:], ars[:, :, 0], inv)
            nc.vector.tensor_scalar_mul(gv[:], ars[:, :, 1], inv)
            nc.vector.tensor_mul(s_t[:], gm[:], gm[:])          # s_t = gm^2 (scratch)
            nc.vector.tensor_sub(gv[:], gv[:], s_t[:])          # gv = E[x^2]-gm^2
            nc.scalar.activation(out=gv[:], in_=gv[:],
                                 func=mybir.ActivationFunctionType.Sqrt,
                                 bias=eps_sb[:], scale=1.0)      # sqrt(var+eps)
            nc.vector.reciprocal(out=gv[:], in_=gv[:])           # rstd
            gidx, bidx = (0, 1) if bn_idx == 1 else (2, 3)
            gam = gb_sb[:, gidx, cocs[0]:cocs[0] + nco]
            bet = gb_sb[:, bidx, cocs[0]:cocs[0] + nco]
            nc.vector.tensor_mul(s_t[:], gv[:], gam)             # s = gamma*rstd
            nc.vector.tensor_mul(t_t[:], gm[:], s_t[:])
            nc.vector.tensor_sub(t_t[:], bet, t_t[:])            # t = beta-gm*s
            return s_t, t_t

        # ---- conv1 -> BN1 stats -> sign -> ba, one coc at a time: coc0's
        # AllReduce + binact hide under coc1's conv1 matmuls
        def binact_pass(coc, s1, t1):
            for img in range(BSH):
                src = out_sb[coc][:, img, :].rearrange("p (r w) -> p r w", w=W)
                # strided [H,W] valid window of the padded image block
                win = ba_im[img][:, coc, 1 + WP: 1 + WP + H * WP]
                win = win.rearrange("p (r w) -> p r w", w=WP)[:, :, 1:1 + W]
                if s1 is None:
                    nc.scalar.activation(out=win, in_=src,
                                         func=mybir.ActivationFunctionType.Sign,
                                         bias=t1[:, 0:1], scale=1.0)
                else:
                    nc.scalar.activation(out=win, in_=src,
                                         func=mybir.ActivationFunctionType.Sign,
                                         bias=t1[:, 0:1], scale=s1[:, 0:1])

        # ---- conv2 -> BN2 stats -> +residual -> sign -> out, one coc at a
        # time: coc0's AllReduce + final passes overlap coc1's conv2 matmuls
        FCK = H * W // 4          # 784-col final chunks

        def final_pass(coc, s2, t2):
            last = None
            for img in range(BSH):
                for rc in range(4):
                    cs = slice(rc * FCK, (rc + 1) * FCK)
                    sl = out_sb[coc][:, img, cs]
                    res = finp.tile([128, FCK], F32, tag="xres", name="xres")
                    nc.sync.dma_start(out=res[:], in_=xr[img, coc, :, cs])
                    # v = (conv*s2 + t2) + x in ONE custom-DVE pass; bf16 is
                    # sign-exact for nonzero values and halves the write BW
                    fv = finbp.tile([128, FCK], BF16, tag="fv", name="fv")
                    nc.vector.affine_then_add(
                        out=fv[:], in0=sl, in1=res[:],
                        scale=s2[:, 0:1], bias=t2[:, 0:1])
                    fin8 = fin8p.tile([128, FCK], BA_DT, tag="fin8", name="fin8")
                    nc.scalar.activation(out=fin8[:], in_=fv[:],
                                         func=mybir.ActivationFunctionType.Sign)
                    nc.scalar.dma_start(out=out[img, coc, :, cs], in_=fin8[:])
                    last = fin8
            return last

        # w2 load: after conv1 emission keeps the startup DMA queues free
        # for w1 + the first x quarters (conv2 starts much later)
        def body(prev_fin8):
            if prev_fin8 is not None:
                # timing-loop serializer: rewrite every xq guard cell (still
                # zero) via an op that reads the previous iteration's last
                # output tile, so this iteration's first matmuls can't start
                # before the previous iteration fully finishes.
                for bset in xq_bufs:
                    for row in bset:
                        for t in row:
                            nc.vector.tensor_scalar(
                                out=t[:, 0:1], in0=prev_fin8[:, 0:1],
                                scalar1=0.0, scalar2=None,
                                op0=mybir.AluOpType.mult)
            for coc in range(NCH):
                st1 = conv_pass(1, (coc,), need_sq=not fast_bn1)
                s1, t1 = sync_bn(st1, f"1{'ab'[coc]}", 1, (coc,))
                binact_pass(coc, s1, t1)
            if prev_fin8 is None:
                nc.scalar.dma_start(out=w2_sb[:], in_=w2[:])
            last = None
            for coc in range(NCH):
                st2 = conv_pass(2, (coc,), need_sq=True)
                s2, t2 = sync_bn(st2, f"2{'ab'[coc]}", 2, (coc,))
                last = final_pass(coc, s2, t2)
            return last

        prev = None
        for _ in range(loops):
            prev = body(prev)

    nc.compile()
    return nc


def build_floor_nc():
    """Same I/O signature, near-zero compute: calibrates dispatch overhead."""
    nc = bacc.Bacc("TRN2", target_bir_lowering=False, debug=False,
                   enable_asserts=False, num_devices=N_CORES)
    nc.dram_tensor("x_hi", [BSH, NCH, 128, FLAT], FP16, kind="ExternalInput")
    nc.dram_tensor("x_lo", [BSH, NCH, 128, FLAT], FP16, kind="ExternalInput")
    xr = nc.dram_tensor("x_res", [BSH, NCH, 128, H * W], F32,
                        kind="ExternalInput").ap()
    nc.dram_tensor("w1t", [NCH, 128, 9, C], FP16, kind="ExternalInput")
    nc.dram_tensor("w2t", [128, NCH, 9, C], BA_DT, kind="ExternalInput")
    nc.dram_tensor("gb", [128, 4, NCH], F32, kind="ExternalInput")
    out = nc.dram_tensor("out", [BSH, NCH, 128, H * W], BA_DT,
                         kind="ExternalOutput").ap()
    with tile.TileContext(nc) as tc, ExitStack() as ctx:
        p = ctx.enter_context(tc.tile_pool(name="p", bufs=2))
        z = ctx.enter_context(tc.tile_pool(name="z", bufs=1))
        zt = z.tile([128, H * W], BA_DT, tag="z", name="z")
        nc.vector.memset(zt[:], 0.0)
        for img in range(BSH):
            for coc in range(NCH):
                t = p.tile([128, H * W], F32, tag="t", name="t")
                nc.sync.dma_start(out=t[:], in_=xr[img, coc])
                nc.sync.dma_start(out=out[img, coc], in_=zt[:])
    nc.compile()
    return nc


# ---------------------------------------------------------------- host side
def _split2(x32):
    """f32 -> fp16 hi + fp16 lo (residual ~2^-24 rel)."""
    hi = x32.astype(np.float16)
    lo = (x32 - hi.astype(np.float32)).astype(np.float16)
    return hi, lo


def preprocess(x, w1, gamma1, beta1, w2, gamma2, beta2):
    """Full inputs -> list of 8 per-core in_maps."""
    x = np.asarray(x, dtype=np.float32)
    xpad = np.zeros((B, C, HP, WP), np.float32)
    xpad[:, :, 1:1 + H, 1:1 + W] = x
    hi, lo = _split2(xpad)

    def wprep(w, dt, scale=1.0, merged=False):
        ws = np.sign(np.asarray(w, np.float32)) * scale  # [co, ci, ky, kx]
        wt = np.ascontiguousarray(ws.transpose(1, 2, 3, 0))  # [ci, ky, kx, co]
        wt = wt.reshape(NCH, 128, 9, C)
        if merged:  # [k, j, tap, co] for DoubleRow (contraction row k+128j)
            wt = np.ascontiguousarray(wt.transpose(1, 0, 2, 3))
        return wt.astype(_np_dt(dt))

    w1t = wprep(w1, FP16)
    w2t = wprep(w2, BA_DT, merged=True)
    gbv = np.stack([np.asarray(a, np.float32) for a in (gamma1, beta1, gamma2, beta2)])
    gb = np.ascontiguousarray(
        gbv.reshape(4, NCH, 128).transpose(2, 0, 1))  # [128, 4, NCH]

    in_maps = []
    for c in range(N_CORES):
        sl = slice(c * BSH, (c + 1) * BSH)
        in_maps.append({
            "x_hi": np.ascontiguousarray(hi[sl]).reshape(BSH, NCH, 128, FLAT),
            "x_lo": np.ascontiguousarray(lo[sl]).reshape(BSH, NCH, 128, FLAT),
            "x_res": np.ascontiguousarray(x[sl]).reshape(BSH, NCH, 128, H * W),
            "w1t": w1t, "w2t": w2t, "gb": gb,
        })
    return in_maps


def postprocess(results):
    outs = [np.asarray(r["out"]).astype(np.float32).reshape(BSH, C, H, W)
            for r in results]
    return np.concatenate(outs, axis=0)


_NC = None
_FAST = None


def get_nc(fast_bn1=True):
    global _NC, _FAST
    if _NC is None or _FAST != fast_bn1:
        _NC = build_nc(fast_bn1=fast_bn1)
        _FAST = fast_bn1
    return _NC


def kernel(**inputs):
    fast = bool(np.all(np.asarray(inputs["beta1"]) == 0.0)
                and np.all(np.asarray(inputs["gamma1"]) > 0.0))
    nc = get_nc(fast_bn1=fast)
    in_maps = preprocess(**inputs)
    res = bass_utils.run_bass_kernel_spmd(nc, in_maps, core_ids=list(range(N_CORES)))
    return postprocess(res.results)



# revision 7
# speedup vs baseline: 1.0314x; 1.0314x over previous
"""Trainium2 Bass kernel for a binary-conv ResNet BasicBlock (training-mode BN).

Reference computation (per nn_BasicBlock_52158082843180):
    out = sign( BN2( conv3x3(sign(BN1(conv3x3(x, sign(w1)))), sign(w2)) ) + x )
with training-mode BatchNorm (batch stats over (N,H,W), biased var, eps=1e-5).

Strategy (8 NeuronCores, data-parallel over batch N=32 -> 4 images/core):
  * conv3x3 as 9 shift-matmuls on TensorE in a 58x58 zero-padded layout.
  * conv1 inputs split on the host into fp16 hi + fp16 lo (exact to ~2^-24);
    conv2 uses e4m3 DoubleRow matmuls (exact +-1 arithmetic).
  * sync-BN via AllReduce of per-core channel sums. When beta1 == 0 and
    gamma1 > 0 (the shipped inputs), sign(BN1(c)) == sign(c - mean), so BN1
    needs NO variance: conv1 skips the whole sum-of-squares pass and the BN1
    AllReduce carries sums only.
  * conv2's sum-of-squares runs on DVE, keeping
    ScalarE free for PSUM evacuation so the PE isn't gated on psum recycling.
  * binary activations live in per-image tiles so conv2's first matmuls start
    as soon as image 0's sign pass lands (not after all four).
  * the final residual+sign tail is chunked (784 cols) and double-pumped:
    residual DMAs prefetch on the SP queue during conv2, the DVE fused
    scale+add writes a bf16 staging tile (sign-exact, less stream BW), ACT
    Sign emits fp8, and output DMAs ride the ACT queue. Stats payload DMAs
    and weight loads also use the ACT queue so they never wait behind bulk
    image streams; the ACT Sqrt table is pre-warmed so the post-AllReduce
    BN-scale chain never stalls on a table load. Host upcasts the output
    to f32.

kernel(**inputs) takes the full unsharded inputs and returns the full output.
"""

import os
import sys

for _p in ("/root/.axon_site/_ro/trn_rl_repo", "/opt/trn_rl_repo"):
    if os.path.isdir(_p) and _p not in sys.path:
        sys.path.append(_p)

import numpy as np
from contextlib import ExitStack

import concourse.bass as bass
import concourse.bacc as bacc
import concourse.tile as tile
from concourse import mybir, bass_utils

# ---------------------------------------------------------------- constants
N_CORES = 8
B, C, H, W = 32, 256, 56, 56
BSH = B // N_CORES            # images per core
HP, WP = H + 2, W + 2         # padded spatial
FLAT = HP * WP                # 3364 padded pixels per image
NCH = C // 128                # channel chunks of 128 (=2)
NTAP = 3                      # 3x3 kernel
NQ = 4                        # quarters per image (14 output rows each)
RQ = H // NQ                  # output rows per quarter (14)
NCK = 2                       # psum chunks per quarter
RCK = RQ // NCK               # output rows per psum chunk (7)
CKW = RCK * WP                # psum chunk width incl. junk cols (406)
VCK = RCK * W                 # valid elements per chunk (392)
QROWS = RQ + 2                # padded input rows needed per quarter (16)
EPS = 1e-5

F32 = mybir.dt.float32
FP16 = mybir.dt.float16
BF16 = mybir.dt.bfloat16
BA_DT = mybir.dt.float8e4    # binary activation storage (+-1 exact)
HCK = CKW // 2               # DoubleRow half-chunk output width (203)

def _np_dt(dt):
    return np.dtype(mybir.dt.np(dt))


# ---------------------------------------------------------------- program
def build_nc(n_cores=N_CORES, fast_bn1=True, loops=1):
    """loops>1 replicates the whole computation serially inside one NEFF
    (each iteration's first reads depend on the previous iteration's last
    output tile) -- used only for timing: per-exec time =
    (T_loops - T_1) / (loops - 1), cancelling dispatch+I/O overhead.
"""
    nc = bacc.Bacc(
        "TRN2",
        target_bir_lowering=False,
        debug=False,
        enable_asserts=False,
        num_devices=n_cores,
    )
    # per-core DRAM I/O
    xh = nc.dram_tensor("x_hi", [BSH, NCH, 128, FLAT], FP16, kind="ExternalInput").ap()
    xl = nc.dram_tensor("x_lo", [BSH, NCH, 128, FLAT], FP16, kind="ExternalInput").ap()
    xr = nc.dram_tensor("x_res", [BSH, NCH, 128, H * W], F32, kind="ExternalInput").ap()
    w1 = nc.dram_tensor("w1t", [NCH, 128, 9, C], FP16, kind="ExternalInput").ap()
    w2 = nc.dram_tensor("w2t", [128, NCH, 9, C], BA_DT, kind="ExternalInput").ap()
    gb = nc.dram_tensor("gb", [128, 4, NCH], F32, kind="ExternalInput").ap()
    out = nc.dram_tensor("out", [BSH, NCH, 128, H * W], BA_DT, kind="ExternalOutput").ap()
    xsp = [xh, xl]

    with tile.TileContext(nc) as tc, ExitStack() as ctx:
        wpool = ctx.enter_context(tc.tile_pool(name="weights", bufs=1))
        big = ctx.enter_context(tc.tile_pool(name="big", bufs=1))
        xqp = ctx.enter_context(tc.tile_pool(name="xq", bufs=1))
        psum = ctx.enter_context(tc.tile_pool(name="psum", bufs=8, space="PSUM"))
        stp = ctx.enter_context(tc.tile_pool(name="stats", bufs=1))
        scrp = ctx.enter_context(tc.tile_pool(name="scr", bufs=2))
        smp = ctx.enter_context(tc.tile_pool(name="small", bufs=1))
        finp = ctx.enter_context(tc.tile_pool(name="fin", bufs=8))
        fin8p = ctx.enter_context(tc.tile_pool(name="fin8", bufs=4))
        finbp = ctx.enter_context(tc.tile_pool(name="finb", bufs=4))
        dram = ctx.enter_context(tc.tile_pool(name="dram", bufs=1, space="DRAM"))

        # ---- persistent tiles
        w1_sb = [wpool.tile([128, 9, C], FP16, tag=f"w1_{c}", name=f"w1_{c}") for c in range(NCH)]
        for c in range(NCH):
            nc.scalar.dma_start(out=w1_sb[c][:], in_=w1[c])
        w2_sb = wpool.tile([128, NCH, 9, C], BA_DT, tag="w2", name="w2")

        # out_sb holds conv1 output (valid pixels only, f32), later reused
        # in-place for conv2 output.
        out_sb = [big.tile([128, BSH, H * W], F32, tag=f"out_{c}", name=f"out_{c}") for c in range(NCH)]
        # binary activations, one padded-layout tile per image so conv2 can
        # start on image 0 while later images' sign passes still run; +1
        # guard element at each end of each cin-chunk plane.
        ba_im = [big.tile([128, NCH, FLAT + 2], BA_DT, tag=f"ba{i}", name=f"ba{i}")
                 for i in range(BSH)]
        for t in ba_im:
            nc.gpsimd.memset(t[:], 0.0)

        # x-quarter staging: 2 manually-rotated buffer sets; only the 2 guard
        # elements (read into junk output columns) need a one-time zero.
        QW = QROWS * WP + 2
        xq_bufs = [[[xqp.tile([128, QW], FP16,
                              tag=f"xq_{b}_{s}_{cic}", name=f"xq_{b}_{s}_{cic}")
                     for cic in range(NCH)] for s in range(2)]
                   for b in range(2)]
        for bset in xq_bufs:
            for row in bset:
                for t in row:
                    nc.vector.memset(t[:, 0:1], 0.0)
                    nc.vector.memset(t[:, QW - 1:QW], 0.0)
        gb_sb = smp.tile([128, 4, NCH], F32, tag="gb", name="gb")
        nc.scalar.dma_start(out=gb_sb[:], in_=gb)
        eps_sb = smp.tile([128, 1], F32, tag="eps", name="eps")
        nc.vector.memset(eps_sb[:], EPS)
        # warm the ACT Sqrt table so the post-AllReduce BN-scale chain does
        # not stall on a lazy ACT_TABLE_LOAD at the first Sqrt use
        warm = smp.tile([128, 1], F32, tag="warm", name="warm")
        nc.scalar.activation(out=warm[:], in_=eps_sb[:],
                             func=mybir.ActivationFunctionType.Sqrt,
                             bias=eps_sb[:], scale=1.0)

        def conv_pass(conv_idx, cocs, need_sq):
            """Emit one conv's matmuls + psum->sbuf copies + per-chunk
            sum (and optionally sumsq) stats for the given output-channel
            chunks. conv1 reads streamed x quarters; conv2 reads ba_im."""
            is1 = conv_idx == 1
            nchunk = BSH * NQ * NCK
            sums = {c: stp.tile([128, nchunk], F32, tag=f"sum_{c}",
                                name=f"sum{conv_idx}_{c}") for c in cocs}
            sqs = {c: stp.tile([128, nchunk], F32, tag=f"sq_{c}",
                               name=f"sq{conv_idx}_{c}") for c in cocs} if need_sq else None
            for img in range(BSH):
                for q in range(NQ):
                    if is1:
                        # stream the 16 padded input rows of this quarter
                        xq = xq_bufs[(img * NQ + q) % 2]
                        for s in range(2):
                            for cic in range(NCH):
                                nc.sync.dma_start(
                                    out=xq[s][cic][:, 1:1 + QROWS * WP],
                                    in_=xsp[s][img, cic, :,
                                               q * RQ * WP: q * RQ * WP + QROWS * WP],
                                )
                    for coc in cocs:
                        pt = [psum.tile([128, CKW], F32, tag="psum", name="pt") for _ in range(NCK)]
                        cosl = slice(coc * 128, (coc + 1) * 128)
                        started = [False] * NCK
                        if is1:
                            # hi/lo fp16 passes: full-chunk matmuls per cin chunk
                            for ky in range(NTAP):
                                for kx in range(NTAP):
                                    tap = ky * NTAP + kx
                                    for cic in range(NCH):
                                        lhsT = w1_sb[cic][:, tap, cosl]
                                        for s in range(2):
                                            for ck in range(NCK):
                                                # xq guard(+1) and tap col(-1) cancel
                                                off = (7 * ck + ky) * WP + kx
                                                last = (tap == 8
                                                        and cic == NCH - 1 and s == 1)
                                                nc.tensor.matmul(
                                                    pt[ck][:], lhsT,
                                                    xq[s][cic][:, off: off + CKW],
                                                    start=not started[ck], stop=last)
                                                started[ck] = True
                        else:
                            # conv2: e4m3 DoubleRow, both cin chunks per MM.
                            # One full-width matmul per (tap, ck): FD=406
                            # keeps DoubleRow above its FD>=256 sweet spot.
                            base = 1 + q * RQ * WP
                            for ky in range(NTAP):
                                for kx in range(NTAP):
                                    tap = ky * NTAP + kx
                                    lhsT = w2_sb[:, :, tap, cosl]
                                    for ck in range(NCK):
                                        off = base + (7 * ck + ky) * WP + kx - 1
                                        nc.tensor.matmul(
                                            pt[ck][:],
                                            lhsT,
                                            ba_im[img][:, :, off: off + CKW],
                                            perf_mode=mybir.MatmulPerfMode.DoubleRow,
                                            start=(tap == 0),
                                            stop=(tap == 8))
                        # evacuate psum (valid cols only); ScalarE copy also
                        # emits the chunk row-sum; DVE computes sum of squares
                        # in one fused pass when the variance is needed
                        for ck in range(NCK):
                            ci = q * NCK + ck
                            sidx = (img * NQ + q) * NCK + ck
                            dst = out_sb[coc][:, img, ci * VCK:(ci + 1) * VCK]
                            dst3 = dst.rearrange("p (r w) -> p r w", w=W)
                            src3 = pt[ck][:].rearrange("p (r w) -> p r w", w=WP)[:, :, 1:1 + W]
                            nc.scalar.activation(
                                out=dst3, in_=src3,
                                func=mybir.ActivationFunctionType.Copy,
                                accum_out=sums[coc][:, sidx:sidx + 1])
                            if need_sq:
                                scr = scrp.tile([128, VCK], F32, tag="scr", name="scr")
                                nc.vector.tensor_mul(scr[:], dst, dst)
                                nc.vector.reduce_sum(sqs[coc][:, sidx:sidx + 1],
                                                     scr[:],
                                                     axis=mybir.AxisListType.X)
            return sums, sqs

        def sync_bn(stats, tag, bn_idx, cocs):
            """AllReduce stats over `cocs`. Full mode: returns per-channel
            (s_t, t_t) scale/bias. Mean-only mode (BN1 fast path): returns
            (None, t_t) with t_t = -global_mean."""
            sums, sqs = stats
            mean_only = sqs is None
            nco = len(cocs)
            width = 1 if mean_only else 2
            pay = smp.tile([128, nco, width], F32, tag=f"pay{tag}", name=f"pay{tag}")
            for i, coc in enumerate(cocs):
                nc.vector.reduce_sum(pay[:, i, 0:1], sums[coc][:],
                                     axis=mybir.AxisListType.X)
                if not mean_only:
                    nc.vector.reduce_sum(pay[:, i, 1:2], sqs[coc][:],
                                         axis=mybir.AxisListType.X)
            cin = dram.tile([128, nco * width], F32, tag=f"cin{tag}", name=f"cin{tag}")
            cout_ = dram.tile([128, nco * width], F32, tag=f"cout{tag}",
                              addr_space="Shared" if n_cores % 2 == 0 else "Local",
                              name=f"ccout{tag}")
            # stats payloads go out on the ACT hwdge queue so they never sit
            # behind bulk image DMAs on the SP queue
            nc.scalar.dma_start(out=cin[:], in_=pay[:].rearrange("p a b -> p (a b)"))
            nc.gpsimd.collective_compute(
                "AllReduce", mybir.AluOpType.add,
                replica_groups=[list(range(n_cores))],
                ins=[cin.opt()], outs=[cout_.opt()],
            )
            ars = smp.tile([128, nco, width], F32, tag=f"ars{tag}", name=f"ars{tag}")
            nc.scalar.dma_start(out=ars[:].rearrange("p a b -> p (a b)"), in_=cout_[:])
            inv = 1.0 / (BSH * n_cores * H * W)
            t_t = smp.tile([128, nco], F32, tag=f"t{tag}", name=f"t{tag}")
            if mean_only:
                # t = -mean; ba = sign(conv - mean)
                nc.vector.tensor_scalar_mul(t_t[:], ars[:, :, 0], -inv)
                return None, t_t
            gm = smp.tile([128, nco], F32, tag=f"gm{tag}", name=f"gm{tag}")
            gv = smp.tile([128, nco], F32, tag=f"gv{tag}", name=f"gv{tag}")
            s_t = smp.tile([128, nco], F32, tag=f"s{tag}", name=f"s{tag}")
            nc.vector.tensor_scalar_mul(gm[:], ars[:, :, 0], inv)
            nc.vector.tensor_scalar_mul(gv[:], ars[:, :, 1], inv)
            nc.vector.tensor_mul(s_t[:], gm[:], gm[:])          # s_t = gm^2 (scratch)
            nc.vector.tensor_sub(gv[:], gv[:], s_t[:])          # gv = E[x^2]-gm^2
            nc.scalar.activation(out=gv[:], in_=gv[:],
                                 func=mybir.ActivationFunctionType.Sqrt,
                                 bias=eps_sb[:], scale=1.0)      # sqrt(var+eps)
            nc.vector.reciprocal(out=gv[:], in_=gv[:])           # rstd
            gidx, bidx = (0, 1) if bn_idx == 1 else (2, 3)
            gam = gb_sb[:, gidx, cocs[0]:cocs[0] + nco]
            bet = gb_sb[:, bidx, cocs[0]:cocs[0] + nco]
            nc.vector.tensor_mul(s_t[:], gv[:], gam)             # s = gamma*rstd
            nc.vector.tensor_mul(t_t[:], gm[:], s_t[:])
            nc.vector.tensor_sub(t_t[:], bet, t_t[:])            # t = beta-gm*s
            return s_t, t_t

        # ---- conv1 -> BN1 stats -> sign -> ba, one coc at a time: coc0's
        # AllReduce + binact hide under coc1's conv1 matmuls
        def binact_pass(coc, s1, t1):
            for img in range(BSH):
                src = out_sb[coc][:, img, :].rearrange("p (r w) -> p r w", w=W)
                # strided [H,W] valid window of the padded image block
                win = ba_im[img][:, coc, 1 + WP: 1 + WP + H * WP]
                win = win.rearrange("p (r w) -> p r w", w=WP)[:, :, 1:1 + W]
                if s1 is None:
                    nc.scalar.activation(out=win, in_=src,
                                         func=mybir.ActivationFunctionType.Sign,
                                         bias=t1[:, 0:1], scale=1.0)
                else:
                    nc.scalar.activation(out=win, in_=src,
                                         func=mybir.ActivationFunctionType.Sign,
                                         bias=t1[:, 0:1], scale=s1[:, 0:1])

        # ---- conv2 -> BN2 stats -> +residual -> sign -> out, one coc at a
        # time: coc0's AllReduce + final passes overlap coc1's conv2 matmuls
        FCK = H * W // 4          # 784-col final chunks

        def final_pass(coc, s2, t2):
            last = None
            for img in range(BSH):
                for rc in range(4):
                    cs = slice(rc * FCK, (rc + 1) * FCK)
                    sl = out_sb[coc][:, img, cs]
                    res = finp.tile([128, FCK], F32, tag="xres", name="xres")
                    nc.sync.dma_start(out=res[:], in_=xr[img, coc, :, cs])
                    # v = (conv*s2 + t2) + x in ONE custom-DVE pass; bf16 is
                    # sign-exact for nonzero values and halves the write BW
                    fv = finbp.tile([128, FCK], BF16, tag="fv", name="fv")
                    nc.vector.affine_then_add(
                        out=fv[:], in0=sl, in1=res[:],
                        scale=s2[:, 0:1], bias=t2[:, 0:1])
                    fin8 = fin8p.tile([128, FCK], BA_DT, tag="fin8", name="fin8")
                    nc.scalar.activation(out=fin8[:], in_=fv[:],
                                         func=mybir.ActivationFunctionType.Sign)
                    nc.scalar.dma_start(out=out[img, coc, :, cs], in_=fin8[:])
                    last = fin8
            return last

        # w2 load: after conv1 emission keeps the startup DMA queues free
        # for w1 + the first x quarters (conv2 starts much later)
        def body(prev_fin8):
            if prev_fin8 is not None:
                # timing-loop serializer: rewrite every xq guard cell (still
                # zero) via an op that reads the previous iteration's last
                # output tile, so this iteration's first matmuls can't start
                # before the previous iteration fully finishes.
                for bset in xq_bufs:
                    for row in bset:
                        for t in row:
                            nc.vector.tensor_scalar(
                                out=t[:, 0:1], in0=prev_fin8[:, 0:1],
                                scalar1=0.0, scalar2=None,
                                op0=mybir.AluOpType.mult)
            for coc in range(NCH):
                st1 = conv_pass(1, (coc,), need_sq=not fast_bn1)
                s1, t1 = sync_bn(st1, f"1{'ab'[coc]}", 1, (coc,))
                binact_pass(coc, s1, t1)
            if prev_fin8 is None:
                nc.scalar.dma_start(out=w2_sb[:], in_=w2[:])
            last = None
            for coc in range(NCH):
                st2 = conv_pass(2, (coc,), need_sq=True)
                s2, t2 = sync_bn(st2, f"2{'ab'[coc]}", 2, (coc,))
                last = final_pass(coc, s2, t2)
            return last

        prev = None
        for _ in range(loops):
            prev = body(prev)

    nc.compile()
    return nc


def build_floor_nc():
    """Same I/O signature, near-zero compute: calibrates dispatch overhead."""
    nc = bacc.Bacc("TRN2", target_bir_lowering=False, debug=False,
                   enable_asserts=False, num_devices=N_CORES)
    nc.dram_tensor("x_hi", [BSH, NCH, 128, FLAT], FP16, kind="ExternalInput")
    nc.dram_tensor("x_lo", [BSH, NCH, 128, FLAT], FP16, kind="ExternalInput")
    xr = nc.dram_tensor("x_res", [BSH, NCH, 128, H * W], F32,
                        kind="ExternalInput").ap()
    nc.dram_tensor("w1t", [NCH, 128, 9, C], FP16, kind="ExternalInput")
    nc.dram_tensor("w2t", [128, NCH, 9, C], BA_DT, kind="ExternalInput")
    nc.dram_tensor("gb", [128, 4, NCH], F32, kind="ExternalInput")
    out = nc.dram_tensor("out", [BSH, NCH, 128, H * W], BA_DT,
                         kind="ExternalOutput").ap()
    with tile.TileContext(nc) as tc, ExitStack() as ctx:
        p = ctx.enter_context(tc.tile_pool(name="p", bufs=2))
        z = ctx.enter_context(tc.tile_pool(name="z", bufs=1))
        zt = z.tile([128, H * W], BA_DT, tag="z", name="z")
        nc.vector.memset(zt[:], 0.0)
        for img in range(BSH):
            for coc in range(NCH):
                t = p.tile([128, H * W], F32, tag="t", name="t")
                nc.sync.dma_start(out=t[:], in_=xr[img, coc])
                nc.sync.dma_start(out=out[img, coc], in_=zt[:])
    nc.compile()
    return nc


# ---------------------------------------------------------------- host side
def _split2(x32):
    """f32 -> fp16 hi + fp16 lo (residual ~2^-24 rel)."""
    hi = x32.astype(np.float16)
    lo = (x32 - hi.astype(np.float32)).astype(np.float16)
    return hi, lo


def preprocess(x, w1, gamma1, beta1, w2, gamma2, beta2):
    """Full inputs -> list of 8 per-core in_maps."""
    x = np.asarray(x, dtype=np.float32)
    xpad = np.zeros((B, C, HP, WP), np.float32)
    xpad[:, :, 1:1 + H, 1:1 + W] = x
    hi, lo = _split2(xpad)

    def wprep(w, dt, scale=1.0, merged=False):
        ws = np.sign(np.asarray(w, np.float32)) * scale  # [co, ci, ky, kx]
        wt = np.ascontiguousarray(ws.transpose(1, 2, 3, 0))  # [ci, ky, kx, co]
        wt = wt.reshape(NCH, 128, 9, C)
        if merged:  # [k, j, tap, co] for DoubleRow (contraction row k+128j)
            wt = np.ascontiguousarray(wt.transpose(1, 0, 2, 3))
        return wt.astype(_np_dt(dt))

    w1t = wprep(w1, FP16)
    w2t = wprep(w2, BA_DT, merged=True)
    gbv = np.stack([np.asarray(a, np.float32) for a in (gamma1, beta1, gamma2, beta2)])
    gb = np.ascontiguousarray(
        gbv.reshape(4, NCH, 128).transpose(2, 0, 1))  # [128, 4, NCH]

    in_maps = []
    for c in range(N_CORES):
        sl = slice(c * BSH, (c + 1) * BSH)
        in_maps.append({
            "x_hi": np.ascontiguousarray(hi[sl]).reshape(BSH, NCH, 128, FLAT),
            "x_lo": np.ascontiguousarray(lo[sl]).reshape(BSH, NCH, 128, FLAT),
            "x_res": np.ascontiguousarray(x[sl]).reshape(BSH, NCH, 128, H * W),
            "w1t": w1t, "w2t": w2t, "gb": gb,
        })
    return in_maps


def postprocess(results):
    outs = [np.asarray(r["out"]).astype(np.float32).reshape(BSH, C, H, W)
            for r in results]
    return np.concatenate(outs, axis=0)


_NC = None
_FAST = None


def get_nc(fast_bn1=True):
    global _NC, _FAST
    if _NC is None or _FAST != fast_bn1:
        _NC = build_nc(fast_bn1=fast_bn1)
        _FAST = fast_bn1
    return _NC


def kernel(**inputs):
    fast = bool(np.all(np.asarray(inputs["beta1"]) == 0.0)
                and np.all(np.asarray(inputs["gamma1"]) > 0.0))
    nc = get_nc(fast_bn1=fast)
    in_maps = preprocess(**inputs)
    res = bass_utils.run_bass_kernel_spmd(nc, in_maps, core_ids=list(range(N_CORES)))
    return postprocess(res.results)

